# revision 13
# baseline (speedup 1.0000x reference)
"""Trainium2 Bass kernel for nn_MinibatchDiscriminator.

reference:
    M = (x @ T).reshape(B, OUT_F, KD)
    norm[i, j, o] = sum_k |M[i,o,k] - M[j,o,k]|
    oX[j, o] = sum_i exp(-norm[i,j,o])
    out = concat(x, oX, axis=1)

Sharding: batch dim of the j-loop across 8 cores. Each core receives a
batch-rotated copy of x^T (so its own 128 j-rows are always M_T columns
0..127 -- one SPMD program serves all cores), computes the full
M_T = (x_rot @ T)^T in [ok, i] layout on the PE.

Symmetry: exp(-norm) is symmetric in (i, j), so each core only computes
i in [0, 640) local (its own diagonal block, neighbours d=1..3, and the
d=4 block which both endpoint cores compute for their own rows). For
d=1..3 the per-(o, i) column sums over the core's j rows are also
accumulated (tile SACC) and redistributed to the i-owning shards during
host-side assembly; the diagonal block contains both (i,j) orders and
the d=4 block is computed by both endpoints, so neither contributes
column sums.

The L1 abs is computed via the relu identity (the TRN2 tensor_scalar ISA
has no float-abs ALU op, but (add, max) is a legal dual-op pair):

    |d| = 2 relu(d) - d  =>  norm = 2 sum_k relu(d_k) - S_i + S_j,
    S[o, i] = sum_k M[i, o, k]

so generation is ONE dual-op DVE tensor_scalar per chunk
((x + (-M_j)) max 0.0, 4x bf16 mode), the -0.5*S_i correction is its
own T0/T1 matmul pair through an identity selector against a constant
-0.5*S tile, and +S_j enters as the exp bias column with scale=-2.
Both S_i and S_j are read from the same bf16 S values, so they cancel
exactly on the diagonal and exp(0)=1 stays exact.

Chunk 3 has only 16 live ok-rows per jsub, so FOUR j-pairs' worth (8 j
values x 16 rows = 128 partitions) are packed into one gen tile,
regenerated once per 4 pr; per-pr selector variants pick the right
32-row band. The per-group scalar column nmt3big is assembled at setup
with 8 small strided SBUF->SBUF DMAs.

Per j-pair (pr, pr+64), one [128, 640] PSUM tile (rows 0:64 = jsub0's
50 o-rows, 64:128 = jsub1's):
  relu(M_T - M_T[:, j])  one dual-op DVE tensor_scalar per chunk; a
                     rotating ~1.1 tiles/pr go to ACT (Relu activation
                     with per-partition bias) to balance the engines
  k-group reduce     PE matmul with a block-ones selector. jsub0 MMs
                     target col-tile T0 (psum rows 0:64), jsub1 MMs
                     target T1 (rows 64:128); chunk-outer interleaving
                     lets the two 128x64 col-tiles stream concurrently.
                     Both jsubs' 16-row chunk-3 are packed in one gen
                     tile (rows 0:16 / 32:48) via a duplicated column
                     block appended to T, keeping full-128-partition APs
                     so the PE never switches tiling mode mid-loop.
  exp + i-sum        single ACT Exp(scale=-2, bias=S_j col) with accum_out
  sacc               transpose contributions matmul, alternating T0/T1
                     psum halves by pr parity (host adds the halves)

x passthrough is done on the host during assembly (the x-part of the
output is the input x unchanged); the device computes only oX.
"""

import ml_dtypes
import numpy as np

import concourse.bacc as bacc
import concourse.bass as bass
import concourse.mybir as mybir
import concourse.tile as tile

B, IN_F, OUT_F, KD = 1024, 1024, 50, 8
OK = OUT_F * KD  # 400
NCORE = 8
JS = B // NCORE  # 128 rows of the batch per core
P = 128
F32 = mybir.dt.float32
BF16 = mybir.dt.bfloat16

IW = 640  # i-range computed per core (5 of 8 blocks, symmetry)
# matmul free-dim slices of the i-range (<=512 each, psum-bank aligned)
HS = [(0, 512), (512, 640)]
TW = 528  # T input padded: cols 400:528 hold T[:, 384:400] tiled 8x

# (jsub, c) generation tiles routed to ACT per pr (rotating; c=0..2 only,
# the packed chunk-3 tile always stays on DVE). 1 tile/pr balances
# ACT (exp + Relu gens) against DVE (fused relu gens).
def _act_pick(pr):
    return {(pr % 2, (pr // 2) % 3)}


def _build_nc():
    nc = bacc.Bacc(
        "TRN2",
        target_bir_lowering=False,
        debug=False,
        num_devices=NCORE,
    )
    xT = nc.dram_tensor("xT", [IN_F, IW], BF16, kind="ExternalInput").ap()
    t_in = nc.dram_tensor("T", [IN_F, TW], BF16, kind="ExternalInput").ap()
    sel_in = nc.dram_tensor("sel", [P, 832], BF16, kind="ExternalInput").ap()
    ox_out = nc.dram_tensor("oxpair", [P, 64], F32, kind="ExternalOutput").ap()
    s_out = nc.dram_tensor("sacc", [P, 384], F32, kind="ExternalOutput").ap()

    with tile.TileContext(nc) as tc:
        with (
            tc.tile_pool(name="const", bufs=1) as cpool,
            tc.tile_pool(name="xtp", bufs=1) as xtpool,
            tc.tile_pool(name="agen", bufs=24) as apool,
            tc.tile_pool(name="psn", bufs=3, space=bass.MemorySpace.PSUM) as psn,
            tc.tile_pool(name="esc", bufs=6) as epool,
        ):
            sel_sb = cpool.tile([P, 832], BF16)
            nc.sync.dma_start(out=sel_sb[:], in_=sel_in)

            # spread input loads over several engine DMA queues so the
            # descriptor generation isn't serialized on one sequencer
            dma_engs = [nc.sync, nc.scalar, nc.gpsimd]
            t_sb = []
            xt_sb = []
            for fc in range(8):
                tt = cpool.tile([P, TW], BF16, tag=f"t{fc}")
                dma_engs[fc % 3].dma_start(
                    out=tt[:], in_=t_in[fc * 128 : (fc + 1) * 128, :]
                )
                t_sb.append(tt)
                xtt = xtpool.tile([P, IW], BF16, tag=f"xt{fc}")
                dma_engs[(fc + 1) % 3].dma_start(
                    out=xtt[:, 0:512], in_=xT[fc * 128 : (fc + 1) * 128, 0:512]
                )
                dma_engs[(fc + 2) % 3].dma_start(
                    out=xtt[:, 512:IW], in_=xT[fc * 128 : (fc + 1) * 128, 512:IW]
                )
                xt_sb.append(xtt)

            # M_T chunks [128, 640] in bf16 (+ negated copy for the scalar
            # operands). bf16 is safe: the smallest cross-pair L1 norm is
            # ~50 while exp(-norm) only registers against the exact self
            # term below norm ~16, so +-2 of bf16 noise cannot surface.
            mtb = [cpool.tile([P, IW], BF16, tag=f"mtb{c}", name=f"mtb{c}") for c in range(3)]
            # chunk 3 source tiled 8x down the partitions: row 16t+r of
            # m3big = M3 ok-row r (t = 2q+s indexes the (q, s) j-slot)
            m3big = cpool.tile([P, IW], BF16, tag="m3", name="m3big")
            # negated fp32 copies OF THE BF16 VALUES (exact upcast) for the
            # per-partition scalar/bias operands, which must be fp32; using
            # raw-fp32 M here would break the exact-zero self term.
            nmt32 = [cpool.tile([P, JS], F32, tag=f"nmt32{c}", name=f"nmt32{c}") for c in range(3)]
            # per-group scalar: nmt3big[16t+r, g] = -M3[r, 4g + q + 64 s]
            nmt3big = cpool.tile([P, 16], F32, tag="nmt3b", name="nmt3big")

            for c in range(3):
                lo = c * 128
                for lo2, hi2 in HS:
                    w2 = hi2 - lo2
                    ps = psn.tile([P, 512], F32, tag="psmt", bufs=2)
                    for fc in range(8):
                        for half in range(2):
                            nc.tensor.matmul(
                                ps[64 * half : 64 * half + 64, 0:w2],
                                t_sb[fc][:, lo + 64 * half : lo + 64 * half + 64],
                                xt_sb[fc][:, lo2:hi2],
                                start=(fc == 0),
                                stop=(fc == 7),
                                skip_group_check=True,
                            )
                    if lo2 == 0:
                        nc.scalar.activation(
                            mtb[c][:, lo2:hi2],
                            ps[:, 0:w2],
                            mybir.ActivationFunctionType.Copy,
                            bias=0.0,
                            scale=1.0,
                        )
                    else:
                        nc.vector.tensor_copy(mtb[c][:, lo2:hi2], ps[:, 0:w2])
                nc.vector.tensor_scalar(
                    nmt32[c][:], mtb[c][:, 0:JS], -1.0, None,
                    op0=mybir.AluOpType.mult,
                )
            # chunk 3 build: T cols 400:528 hold T3 tiled 8x
            for lo2, hi2 in HS:
                w2 = hi2 - lo2
                ps = psn.tile([P, 512], F32, tag="psmt", bufs=2)
                for fc in range(8):
                    for half in range(2):
                        nc.tensor.matmul(
                            ps[64 * half : 64 * half + 64, 0:w2],
                            t_sb[fc][:, 400 + 64 * half : 400 + 64 * half + 64],
                            xt_sb[fc][:, lo2:hi2],
                            start=(fc == 0),
                            stop=(fc == 7),
                            skip_group_check=True,
                        )
                if lo2 == 0:
                    nc.scalar.activation(
                        m3big[:, lo2:hi2],
                        ps[:, 0:w2],
                        mybir.ActivationFunctionType.Copy,
                        bias=0.0,
                        scale=1.0,
                    )
                else:
                    nc.vector.tensor_copy(m3big[:, lo2:hi2], ps[:, 0:w2])
            # nmt3big[16t+r, g] = -M3[r, 4g + q + 64 s], t = 2q + s:
            # negate once, then 8 strided partition-shift DMAs
            negm3 = cpool.tile([16, JS], F32, tag="negm3", name="negm3")
            nc.vector.tensor_scalar(
                negm3[:], m3big[0:16, 0:JS], -1.0, None,
                op0=mybir.AluOpType.mult,
            )
            for q in range(4):
                for s in range(2):
                    t = 2 * q + s
                    nc.sync.dma_start(
                        out=nmt3big[16 * t : 16 * t + 16, 0:16],
                        in_=negm3[0:16, q + 64 * s : q + 64 * s + 61 : 4],
                    )

            # S[o, i] = sum_k M[i, o, k] via the selector matmuls (the
            # q=0/T0 chunk-3 selector picks m3big rows 0:16 = M3 once)
            psS = psn.tile([P, IW], F32, tag="psn", name="psS")
            for lo2, hi2 in HS:
                for ci, srct in enumerate([mtb[0], mtb[1], mtb[2], m3big]):
                    wsel = sel_sb[:, 64 * ci : 64 * ci + 64] if ci < 3 else sel_sb[:, 192:256]
                    nc.tensor.matmul(
                        psS[0:64, lo2:hi2],
                        wsel,
                        srct[:, lo2:hi2],
                        start=(ci == 0),
                        stop=(ci == 3),
                    )

            # constant -0.5*S tile (bf16) for the identity-selector matmul
            # pair; rows 50:128 zero
            sneg = cpool.tile([P, IW], BF16, tag="sneg", name="sneg")
            nc.vector.memset(sneg[:], 0.0)
            for lo2, hi2 in HS:
                nc.vector.tensor_scalar(
                    sneg[0:50, lo2:hi2], psS[0:50, lo2:hi2], -0.5, None,
                    op0=mybir.AluOpType.mult,
                )
            # exp bias column: sjcol[r, pr] = 2 * sneg_bf16[o(r), j(r, pr)]
            # (= -S_bf[o, j]); rows 64:114 need the j+64 columns, moved
            # across partitions with a small SBUF->SBUF DMA
            sj2 = cpool.tile([64, JS], F32, tag="sj2", name="sj2")
            sjcol = cpool.tile([P, 64], F32, tag="sjcol", name="sjcol")
            nc.vector.memset(sjcol[:], 0.0)
            nc.vector.tensor_scalar(
                sj2[0:50, :], sneg[0:50, 0:JS], 2.0, None,
                op0=mybir.AluOpType.mult,
            )
            nc.sync.dma_start(out=sjcol[0:50, :], in_=sj2[0:50, 0:64])
            nc.sync.dma_start(out=sjcol[64:114, :], in_=sj2[0:50, 64:128])
            # two alternating 4-pr-group chunk-3 tiles (8 j-slots each)
            a3t = [
                cpool.tile([P, IW], BF16, tag=f"a3_{par}", name=f"a3_{par}")
                for par in range(2)
            ]

            oxacc = cpool.tile([P, 64], F32)
            psum_s = psn.tile([P, 384], F32, tag="psmt", bufs=2, name="psum_s")

            add = mybir.AluOpType.add
            mx = mybir.AluOpType.max

            # main loop: j-pairs (pr, pr+64) share one [128, IW] psum
            # tile. Software-pipelined 1 deep: generation for pr is
            # emitted BEFORE the matmuls/exp of pr-1 so the ACT-assigned
            # relu gens sit ahead of the stalling exp in ACT's FIFO
            # (breaking the exp -> relu -> matmul -> exp critical cycle).
            def emit_gens(pr):
                act_set = _act_pick(pr)
                av = [[None] * 3 for _ in range(2)]
                for jsub in range(2):
                    j = pr + 64 * jsub
                    for c in range(3):
                        a = apool.tile([P, IW], BF16, tag="A")
                        if (jsub, c) in act_set:
                            nc.scalar.activation(
                                a[:],
                                mtb[c][:],
                                mybir.ActivationFunctionType.Relu,
                                bias=nmt32[c][:, j : j + 1],
                                scale=1.0,
                            )
                        else:
                            nc.vector.tensor_scalar(
                                a[:], mtb[c][:], nmt32[c][:, j : j + 1], 0.0,
                                op0=add, op1=mx,
                            )
                        av[jsub][c] = a
                g = pr // 4
                a3 = a3t[g % 2]
                if pr % 4 == 0:
                    nc.vector.tensor_scalar(
                        a3[:], m3big[:], nmt3big[:, g : g + 1], 0.0,
                        op0=add, op1=mx,
                    )
                return av, a3

            def emit_reduce(pr, av, a3):
                # k-group reduce: chunk-outer, jsub-inner so consecutive
                # matmuls land on alternating 128x64 col-tiles T0/T1 and
                # stream concurrently. The c=3 matmul also applies the
                # -0.5*S_i correction through the identity selector block.
                ps = psn.tile([P, IW], F32, tag="psn")
                q = pr % 4
                for lo, hi in HS:
                    for c in range(5):
                        for jsub in range(2):
                            r0 = 64 * jsub
                            if c < 3:
                                w = sel_sb[:, 64 * c : 64 * (c + 1)]
                                rhs = av[jsub][c]
                            elif c == 3:
                                w = sel_sb[:, 192 + 128 * q + 64 * jsub : 256 + 128 * q + 64 * jsub]
                                rhs = a3
                            else:
                                w = sel_sb[:, 704:768]
                                rhs = sneg
                            nc.tensor.matmul(
                                ps[r0 : r0 + 64, lo:hi],
                                w,
                                rhs[:, lo:hi],
                                start=(c == 0),
                                stop=(c == 4),
                                skip_group_check=True,
                            )
                e = epool.tile([P, IW], BF16, tag="E")
                nc.scalar.activation(
                    e[:],
                    ps[:],
                    mybir.ActivationFunctionType.Exp,
                    bias=sjcol[:, pr : pr + 1],
                    scale=-2.0,
                    accum_out=oxacc[:, pr : pr + 1],
                )
                # transpose contributions for the d=1..3 i-blocks: fold the
                # two j-halves and accumulate over all pairs on the PE.
                # Alternate psum halves (col-tiles T0/T1) by pr parity so
                # the sacc matmul overlaps the other tile's chunk chain.
                s0 = 64 * (pr % 2)
                nc.tensor.matmul(
                    psum_s[s0 : s0 + 64, :],
                    sel_sb[:, 768:832],
                    e[:, 128:512],
                    start=(pr < 2),
                    stop=(pr >= 62),
                    skip_group_check=True,
                )

            prev = None
            for pr in range(65):
                cur = emit_gens(pr) if pr < 64 else None
                if prev is not None:
                    emit_reduce(pr - 1, *prev)
                prev = cur

            sacc_sb = cpool.tile([P, 384], F32)
            nc.vector.tensor_copy(sacc_sb[:], psum_s[:])
            nc.sync.dma_start(out=ox_out, in_=oxacc[:])
            nc.sync.dma_start(out=s_out, in_=sacc_sb[:])

    nc.compile()
    return nc


_NC = None


def _get_nc():
    global _NC
    if _NC is None:
        _NC = _build_nc()
    return _NC


def _make_in_maps(x, t):
    x = np.ascontiguousarray(np.asarray(x, dtype=np.float32))
    t16 = np.asarray(t, dtype=np.float32).astype(ml_dtypes.bfloat16)
    tpad = np.zeros((IN_F, TW), dtype=ml_dtypes.bfloat16)
    tpad[:, 0:OK] = t16
    tpad[:, 400:528] = np.tile(t16[:, 384:400], (1, 8))
    tpad = np.ascontiguousarray(tpad)
    xtg = np.ascontiguousarray(x.T.astype(ml_dtypes.bfloat16))
    sel = np.zeros((P, 832), dtype=ml_dtypes.bfloat16)
    # chunks 0..2: chunk c maps partition p (= ok - 128c) to o-row
    # 16c + p // KD of the 64-row psum block
    for c in range(3):
        for g in range(16):
            sel[g * KD : (g + 1) * KD, 64 * c + 16 * c + g] = 1.0
    # chunk-3 variants: for pr%4 == q, the jsub-s matmul picks m3big's
    # 16-row band at 32q + 16s and maps its two o-groups to rows 48:50
    for q in range(4):
        for s in range(2):
            base = 32 * q + 16 * s
            for g in range(2):
                sel[base + g * KD : base + (g + 1) * KD, 192 + 128 * q + 64 * s + 48 + g] = 1.0
    # identity block for the -0.5*S_i pair
    for o in range(OUT_F):
        sel[o, 704 + o] = 1.0
    # sacc fold: psum partition p -> column p % 64
    for pp in range(P):
        sel[pp, 768 + (pp % 64)] = 1.0
    in_maps = []
    for c in range(NCORE):
        in_maps.append(
            {
                "xT": np.ascontiguousarray(np.roll(xtg, -c * JS, axis=1)[:, :IW]),
                "T": tpad,
                "sel": sel,
            }
        )
    return in_maps


def _assemble(x, results):
    x = np.asarray(x, dtype=np.float32)
    out = np.empty((B, IN_F + OUT_F), dtype=np.float32)
    out[:, :IN_F] = x
    oX = np.zeros((B, OUT_F), dtype=np.float32)
    for c in range(NCORE):
        r = results[c]
        rows = slice(c * JS, (c + 1) * JS)
        oxp = r["oxpair"]  # [128, 64]: rows 0:50 -> j=pr, rows 64:114 -> j=pr+64
        oX[rows] += np.concatenate(
            [oxp[0:OUT_F, :].T, oxp[64 : 64 + OUT_F, :].T], axis=0
        )
        # transpose contributions: sacc[(parity, o), t] sums exp terms over
        # this core's even/odd j rows for local i = 128 + t (d=1..3 blocks)
        s = r["sacc"]
        s50 = (s[0:OUT_F, :] + s[64 : 64 + OUT_F, :]).T  # [384, 50]
        g0 = (c + 1) * JS
        for blk in range(3):
            gs = (g0 + blk * JS) % B
            oX[gs : gs + JS] += s50[blk * JS : (blk + 1) * JS]
    out[:, IN_F:] = oX
    return out


def kernel(x, T):
    from concourse.bass_utils import run_bass_kernel_spmd

    nc = _get_nc()
    in_maps = _make_in_maps(x, T)
    res = run_bass_kernel_spmd(nc, in_maps, core_ids=list(range(NCORE)))
    return _assemble(x, res.results)


def _ensure_ntff_hook():
    """The agent image's antenv lacks axon_hooks; synthesize it from the
    ctypes NTFF driver in trn_agent_boot so trace=True works."""
    import sys
    import types

    try:
        from antenv.axon_hooks import get_axon_ntff_profile_hook  # noqa: F401

        return
    except ImportError:
        pass
    from trn_agent_boot.trn_boot import _ntff_profile_via_ctypes

    hook = _ntff_profile_via_ctypes("/opt/axon/libaxon_pjrt.so")
    mod = types.ModuleType("antenv.axon_hooks")
    mod.get_axon_ntff_profile_hook = lambda: hook
    mod.set_axon_ntff_profile_hook = lambda h: None
    sys.modules["antenv.axon_hooks"] = mod


def kernel_profiled(x, T, tmpdir=None):
    """Same as kernel() but with NTFF tracing; returns (out, exec_time_ns)."""
    import concourse.bass_utils as bu

    _ensure_ntff_hook()
    bu.upload_artifacts = lambda d: d  # no S3 in this container

    nc = _get_nc()
    in_maps = _make_in_maps(x, T)
    res = bu.run_bass_kernel_spmd(
        nc, in_maps, core_ids=list(range(NCORE)), trace=True, tmpdir=tmpdir
    )
    return _assemble(x, res.results), res.exec_time_ns


# revision 14
# speedup vs baseline: 1.0249x; 1.0249x over previous
"""Trainium2 Bass kernel for nn_MinibatchDiscriminator.

reference:
    M = (x @ T).reshape(B, OUT_F, KD)
    norm[i, j, o] = sum_k |M[i,o,k] - M[j,o,k]|
    oX[j, o] = sum_i exp(-norm[i,j,o])
    out = concat(x, oX, axis=1)

Sharding: batch dim of the j-loop across 8 cores. Each core receives a
batch-rotated copy of x^T (so its own 128 j-rows are always M_T columns
0..127 -- one SPMD program serves all cores), computes the full
M_T = (x_rot @ T)^T in [ok, i] layout on the PE.

Symmetry: exp(-norm) is symmetric in (i, j), so each core only computes
i in [0, 640) local (its own diagonal block, neighbours d=1..3, and the
d=4 block which both endpoint cores compute for their own rows). For
d=1..3 the per-(o, i) column sums over the core's j rows are also
accumulated (tile SACC) and redistributed to the i-owning shards during
host-side assembly; the diagonal block contains both (i,j) orders and
the d=4 block is computed by both endpoints, so neither contributes
column sums.

The L1 abs is computed via the relu identity (the TRN2 tensor_scalar ISA
has no float-abs ALU op, but (add, max) is a legal dual-op pair):

    |d| = 2 relu(d) - d  =>  norm = 2 sum_k relu(d_k) - S_i + S_j,
    S[o, i] = sum_k M[i, o, k]

so generation is ONE dual-op DVE tensor_scalar per chunk
((x + (-M_j)) max 0.0, 4x bf16 mode), the -0.5*S_i correction is its
own T0/T1 matmul pair through an identity selector against a constant
-0.5*S tile, and +S_j enters as the exp bias column with scale=-2.
Both S_i and S_j are read from the same bf16 S values, so they cancel
exactly on the diagonal and exp(0)=1 stays exact.

Chunk 3 has only 16 live ok-rows per jsub, so FOUR j-pairs' worth (8 j
values x 16 rows = 128 partitions) are packed into one gen tile,
regenerated once per 4 pr; per-pr selector variants pick the right
32-row band. The per-group scalar column nmt3big is assembled at setup
with 8 small strided SBUF->SBUF DMAs.

Per j-pair (pr, pr+64), one [128, 640] PSUM tile (rows 0:64 = jsub0's
50 o-rows, 64:128 = jsub1's):
  relu(M_T - M_T[:, j])  one dual-op DVE tensor_scalar per chunk; a
                     rotating ~1.1 tiles/pr go to ACT (Relu activation
                     with per-partition bias) to balance the engines
  k-group reduce     PE matmul with a block-ones selector. jsub0 MMs
                     target col-tile T0 (psum rows 0:64), jsub1 MMs
                     target T1 (rows 64:128); chunk-outer interleaving
                     lets the two 128x64 col-tiles stream concurrently.
                     Both jsubs' 16-row chunk-3 are packed in one gen
                     tile (rows 0:16 / 32:48) via a duplicated column
                     block appended to T, keeping full-128-partition APs
                     so the PE never switches tiling mode mid-loop.
  exp + i-sum        single ACT Exp(scale=-2, bias=S_j col) with accum_out
  sacc               transpose contributions matmul, alternating T0/T1
                     psum halves by pr parity (host adds the halves)

x passthrough is done on the host during assembly (the x-part of the
output is the input x unchanged); the device computes only oX.
"""

import ml_dtypes
import numpy as np

import concourse.bacc as bacc
import concourse.bass as bass
import concourse.mybir as mybir
import concourse.tile as tile

B, IN_F, OUT_F, KD = 1024, 1024, 50, 8
OK = OUT_F * KD  # 400
NCORE = 8
JS = B // NCORE  # 128 rows of the batch per core
P = 128
F32 = mybir.dt.float32
BF16 = mybir.dt.bfloat16

IW = 640  # i-range computed per core (5 of 8 blocks, symmetry)
# matmul free-dim slices of the i-range (<=512 each, psum-bank aligned)
HS = [(0, 512), (512, 640)]
TW = 528  # T input padded: cols 400:528 hold T[:, 384:400] tiled 8x

# (jsub, c) generation tiles routed to ACT per pr (rotating; c=0..2 only,
# the packed chunk-3 tile always stays on DVE). 1 tile/pr balances
# ACT (exp + Relu gens) against DVE (fused relu gens).
def _act_pick(pr):
    return {(pr % 2, 2)}


def _build_nc():
    nc = bacc.Bacc(
        "TRN2",
        target_bir_lowering=False,
        debug=False,
        num_devices=NCORE,
    )
    xT = nc.dram_tensor("xT", [IN_F, IW], BF16, kind="ExternalInput").ap()
    t_in = nc.dram_tensor("T", [IN_F, TW], BF16, kind="ExternalInput").ap()
    sel_in = nc.dram_tensor("sel", [P, 832], BF16, kind="ExternalInput").ap()
    ox_out = nc.dram_tensor("oxpair", [P, 64], F32, kind="ExternalOutput").ap()
    s_out = nc.dram_tensor("sacc", [P, 384], F32, kind="ExternalOutput").ap()

    with tile.TileContext(nc) as tc:
        with (
            tc.tile_pool(name="const", bufs=1) as cpool,
            tc.tile_pool(name="xtp", bufs=1) as xtpool,
            tc.tile_pool(name="agen", bufs=24) as apool,
            tc.tile_pool(name="psn", bufs=3, space=bass.MemorySpace.PSUM) as psn,
            tc.tile_pool(name="esc", bufs=6) as epool,
        ):
            sel_sb = cpool.tile([P, 832], BF16)
            nc.sync.dma_start(out=sel_sb[:], in_=sel_in)

            # spread input loads over several engine DMA queues so the
            # descriptor generation isn't serialized on one sequencer
            dma_engs = [nc.sync, nc.scalar, nc.gpsimd]
            t_sb = []
            xt_sb = []
            for fc in range(8):
                tt = cpool.tile([P, TW], BF16, tag=f"t{fc}")
                dma_engs[fc % 3].dma_start(
                    out=tt[:], in_=t_in[fc * 128 : (fc + 1) * 128, :]
                )
                t_sb.append(tt)
                xtt = xtpool.tile([P, IW], BF16, tag=f"xt{fc}")
                dma_engs[(fc + 1) % 3].dma_start(
                    out=xtt[:, 0:512], in_=xT[fc * 128 : (fc + 1) * 128, 0:512]
                )
                dma_engs[(fc + 2) % 3].dma_start(
                    out=xtt[:, 512:IW], in_=xT[fc * 128 : (fc + 1) * 128, 512:IW]
                )
                xt_sb.append(xtt)

            # M_T chunks [128, 640] in bf16 (+ negated copy for the scalar
            # operands). bf16 is safe: the smallest cross-pair L1 norm is
            # ~50 while exp(-norm) only registers against the exact self
            # term below norm ~16, so +-2 of bf16 noise cannot surface.
            mtb = [cpool.tile([P, IW], BF16, tag=f"mtb{c}", name=f"mtb{c}") for c in range(3)]
            # chunk 3 source tiled 8x down the partitions: row 16t+r of
            # m3big = M3 ok-row r (t = 2q+s indexes the (q, s) j-slot)
            m3big = cpool.tile([P, IW], BF16, tag="m3", name="m3big")
            # negated fp32 copies OF THE BF16 VALUES (exact upcast) for the
            # per-partition scalar/bias operands, which must be fp32; using
            # raw-fp32 M here would break the exact-zero self term.
            nmt32 = [cpool.tile([P, JS], F32, tag=f"nmt32{c}", name=f"nmt32{c}") for c in range(3)]
            # per-group scalar: nmt3big[16t+r, g] = -M3[r, 4g + q + 64 s]
            nmt3big = cpool.tile([P, 16], F32, tag="nmt3b", name="nmt3big")

            for c in range(3):
                lo = c * 128
                for lo2, hi2 in HS:
                    w2 = hi2 - lo2
                    ps = psn.tile([P, 512], F32, tag="psmt", bufs=2)
                    for fc in range(8):
                        for half in range(2):
                            nc.tensor.matmul(
                                ps[64 * half : 64 * half + 64, 0:w2],
                                t_sb[fc][:, lo + 64 * half : lo + 64 * half + 64],
                                xt_sb[fc][:, lo2:hi2],
                                start=(fc == 0),
                                stop=(fc == 7),
                                skip_group_check=True,
                            )
                    if lo2 == 0:
                        nc.scalar.activation(
                            mtb[c][:, lo2:hi2],
                            ps[:, 0:w2],
                            mybir.ActivationFunctionType.Copy,
                            bias=0.0,
                            scale=1.0,
                        )
                    else:
                        nc.vector.tensor_copy(mtb[c][:, lo2:hi2], ps[:, 0:w2])
                nc.vector.tensor_scalar(
                    nmt32[c][:], mtb[c][:, 0:JS], -1.0, None,
                    op0=mybir.AluOpType.mult,
                )
            # chunk 3 build: T cols 400:528 hold T3 tiled 8x
            for lo2, hi2 in HS:
                w2 = hi2 - lo2
                ps = psn.tile([P, 512], F32, tag="psmt", bufs=2)
                for fc in range(8):
                    for half in range(2):
                        nc.tensor.matmul(
                            ps[64 * half : 64 * half + 64, 0:w2],
                            t_sb[fc][:, 400 + 64 * half : 400 + 64 * half + 64],
                            xt_sb[fc][:, lo2:hi2],
                            start=(fc == 0),
                            stop=(fc == 7),
                            skip_group_check=True,
                        )
                if lo2 == 0:
                    nc.scalar.activation(
                        m3big[:, lo2:hi2],
                        ps[:, 0:w2],
                        mybir.ActivationFunctionType.Copy,
                        bias=0.0,
                        scale=1.0,
                    )
                else:
                    nc.vector.tensor_copy(m3big[:, lo2:hi2], ps[:, 0:w2])
            # nmt3big[16t+r, g] = -M3[r, 4g + q + 64 s], t = 2q + s:
            # negate once, then 8 strided partition-shift DMAs
            negm3 = cpool.tile([16, JS], F32, tag="negm3", name="negm3")
            nc.vector.tensor_scalar(
                negm3[:], m3big[0:16, 0:JS], -1.0, None,
                op0=mybir.AluOpType.mult,
            )
            for q in range(4):
                for s in range(2):
                    t = 2 * q + s
                    nc.sync.dma_start(
                        out=nmt3big[16 * t : 16 * t + 16, 0:16],
                        in_=negm3[0:16, q + 64 * s : q + 64 * s + 61 : 4],
                    )

            # S[o, i] = sum_k M[i, o, k] via the selector matmuls (the
            # q=0/T0 chunk-3 selector picks m3big rows 0:16 = M3 once)
            psS = psn.tile([P, IW], F32, tag="psn", name="psS")
            for lo2, hi2 in HS:
                for ci, srct in enumerate([mtb[0], mtb[1], mtb[2], m3big]):
                    wsel = sel_sb[:, 64 * ci : 64 * ci + 64] if ci < 3 else sel_sb[:, 192:256]
                    nc.tensor.matmul(
                        psS[0:64, lo2:hi2],
                        wsel,
                        srct[:, lo2:hi2],
                        start=(ci == 0),
                        stop=(ci == 3),
                    )

            # constant -0.5*S tile (bf16) for the identity-selector matmul
            # pair; rows 50:128 zero
            sneg = cpool.tile([P, IW], BF16, tag="sneg", name="sneg")
            nc.vector.memset(sneg[:], 0.0)
            for lo2, hi2 in HS:
                nc.vector.tensor_scalar(
                    sneg[0:50, lo2:hi2], psS[0:50, lo2:hi2], -0.5, None,
                    op0=mybir.AluOpType.mult,
                )
            # exp bias column: sjcol[r, pr] = 2 * sneg_bf16[o(r), j(r, pr)]
            # (= -S_bf[o, j]); rows 64:114 need the j+64 columns, moved
            # across partitions with a small SBUF->SBUF DMA
            sj2 = cpool.tile([64, JS], F32, tag="sj2", name="sj2")
            sjcol = cpool.tile([P, 64], F32, tag="sjcol", name="sjcol")
            nc.vector.memset(sjcol[:], 0.0)
            nc.vector.tensor_scalar(
                sj2[0:50, :], sneg[0:50, 0:JS], 2.0, None,
                op0=mybir.AluOpType.mult,
            )
            nc.sync.dma_start(out=sjcol[0:50, :], in_=sj2[0:50, 0:64])
            nc.sync.dma_start(out=sjcol[64:114, :], in_=sj2[0:50, 64:128])
            # two alternating 4-pr-group chunk-3 tiles (8 j-slots each)
            a3t = [
                cpool.tile([P, IW], BF16, tag=f"a3_{par}", name=f"a3_{par}")
                for par in range(2)
            ]

            oxacc = cpool.tile([P, 64], F32)
            psum_s = psn.tile([P, 384], F32, tag="psmt", bufs=2, name="psum_s")

            add = mybir.AluOpType.add
            mx = mybir.AluOpType.max

            # main loop: j-pairs (pr, pr+64) share one [128, IW] psum
            # tile. Software-pipelined 1 deep: generation for pr is
            # emitted BEFORE the matmuls/exp of pr-1 so the ACT-assigned
            # relu gens sit ahead of the stalling exp in ACT's FIFO
            # (breaking the exp -> relu -> matmul -> exp critical cycle).
            def emit_gens(pr):
                act_set = _act_pick(pr)
                av = [[None] * 3 for _ in range(2)]
                for jsub in range(2):
                    j = pr + 64 * jsub
                    for c in range(3):
                        a = apool.tile([P, IW], BF16, tag="A")
                        if (jsub, c) in act_set:
                            nc.scalar.activation(
                                a[:],
                                mtb[c][:],
                                mybir.ActivationFunctionType.Relu,
                                bias=nmt32[c][:, j : j + 1],
                                scale=1.0,
                            )
                        else:
                            nc.vector.tensor_scalar(
                                a[:], mtb[c][:], nmt32[c][:, j : j + 1], 0.0,
                                op0=add, op1=mx,
                            )
                        av[jsub][c] = a
                g = pr // 4
                a3 = a3t[g % 2]
                if pr % 4 == 0:
                    nc.vector.tensor_scalar(
                        a3[:], m3big[:], nmt3big[:, g : g + 1], 0.0,
                        op0=add, op1=mx,
                    )
                return av, a3

            def emit_reduce(pr, av, a3):
                # k-group reduce: chunk-outer, jsub-inner so consecutive
                # matmuls land on alternating 128x64 col-tiles T0/T1 and
                # stream concurrently. The c=3 matmul also applies the
                # -0.5*S_i correction through the identity selector block.
                ps = psn.tile([P, IW], F32, tag="psn")
                q = pr % 4
                for lo, hi in HS:
                    for ci, c in enumerate([4, 0, 1, 3, 2]):
                        for jsub in range(2):
                            r0 = 64 * jsub
                            if c < 3:
                                w = sel_sb[:, 64 * c : 64 * (c + 1)]
                                rhs = av[jsub][c]
                            elif c == 3:
                                w = sel_sb[:, 192 + 128 * q + 64 * jsub : 256 + 128 * q + 64 * jsub]
                                rhs = a3
                            else:
                                w = sel_sb[:, 704:768]
                                rhs = sneg
                            nc.tensor.matmul(
                                ps[r0 : r0 + 64, lo:hi],
                                w,
                                rhs[:, lo:hi],
                                start=(ci == 0),
                                stop=(ci == 4),
                                skip_group_check=True,
                            )
                e = epool.tile([P, IW], BF16, tag="E")
                nc.scalar.activation(
                    e[:],
                    ps[:],
                    mybir.ActivationFunctionType.Exp,
                    bias=sjcol[:, pr : pr + 1],
                    scale=-2.0,
                    accum_out=oxacc[:, pr : pr + 1],
                )
                # transpose contributions for the d=1..3 i-blocks: fold the
                # two j-halves and accumulate over all pairs on the PE.
                # Alternate psum halves (col-tiles T0/T1) by pr parity so
                # the sacc matmul overlaps the other tile's chunk chain.
                s0 = 64 * (pr % 2)
                nc.tensor.matmul(
                    psum_s[s0 : s0 + 64, :],
                    sel_sb[:, 768:832],
                    e[:, 128:512],
                    start=(pr < 2),
                    stop=(pr >= 62),
                    skip_group_check=True,
                )

            prev = None
            for pr in range(65):
                cur = emit_gens(pr) if pr < 64 else None
                if prev is not None:
                    emit_reduce(pr - 1, *prev)
                prev = cur

            sacc_sb = cpool.tile([P, 384], F32)
            nc.vector.tensor_copy(sacc_sb[:], psum_s[:])
            nc.sync.dma_start(out=ox_out, in_=oxacc[:])
            nc.sync.dma_start(out=s_out, in_=sacc_sb[:])

    nc.compile()
    return nc


_NC = None


def _get_nc():
    global _NC
    if _NC is None:
        _NC = _build_nc()
    return _NC


def _make_in_maps(x, t):
    x = np.ascontiguousarray(np.asarray(x, dtype=np.float32))
    t16 = np.asarray(t, dtype=np.float32).astype(ml_dtypes.bfloat16)
    tpad = np.zeros((IN_F, TW), dtype=ml_dtypes.bfloat16)
    tpad[:, 0:OK] = t16
    tpad[:, 400:528] = np.tile(t16[:, 384:400], (1, 8))
    tpad = np.ascontiguousarray(tpad)
    xtg = np.ascontiguousarray(x.T.astype(ml_dtypes.bfloat16))
    sel = np.zeros((P, 832), dtype=ml_dtypes.bfloat16)
    # chunks 0..2: chunk c maps partition p (= ok - 128c) to o-row
    # 16c + p // KD of the 64-row psum block
    for c in range(3):
        for g in range(16):
            sel[g * KD : (g + 1) * KD, 64 * c + 16 * c + g] = 1.0
    # chunk-3 variants: for pr%4 == q, the jsub-s matmul picks m3big's
    # 16-row band at 32q + 16s and maps its two o-groups to rows 48:50
    for q in range(4):
        for s in range(2):
            base = 32 * q + 16 * s
            for g in range(2):
                sel[base + g * KD : base + (g + 1) * KD, 192 + 128 * q + 64 * s + 48 + g] = 1.0
    # identity block for the -0.5*S_i pair
    for o in range(OUT_F):
        sel[o, 704 + o] = 1.0
    # sacc fold: psum partition p -> column p % 64
    for pp in range(P):
        sel[pp, 768 + (pp % 64)] = 1.0
    in_maps = []
    for c in range(NCORE):
        in_maps.append(
            {
                "xT": np.ascontiguousarray(np.roll(xtg, -c * JS, axis=1)[:, :IW]),
                "T": tpad,
                "sel": sel,
            }
        )
    return in_maps


def _assemble(x, results):
    x = np.asarray(x, dtype=np.float32)
    out = np.empty((B, IN_F + OUT_F), dtype=np.float32)
    out[:, :IN_F] = x
    oX = np.zeros((B, OUT_F), dtype=np.float32)
    for c in range(NCORE):
        r = results[c]
        rows = slice(c * JS, (c + 1) * JS)
        oxp = r["oxpair"]  # [128, 64]: rows 0:50 -> j=pr, rows 64:114 -> j=pr+64
        oX[rows] += np.concatenate(
            [oxp[0:OUT_F, :].T, oxp[64 : 64 + OUT_F, :].T], axis=0
        )
        # transpose contributions: sacc[(parity, o), t] sums exp terms over
        # this core's even/odd j rows for local i = 128 + t (d=1..3 blocks)
        s = r["sacc"]
        s50 = (s[0:OUT_F, :] + s[64 : 64 + OUT_F, :]).T  # [384, 50]
        g0 = (c + 1) * JS
        for blk in range(3):
            gs = (g0 + blk * JS) % B
            oX[gs : gs + JS] += s50[blk * JS : (blk + 1) * JS]
    out[:, IN_F:] = oX
    return out


def kernel(x, T):
    from concourse.bass_utils import run_bass_kernel_spmd

    nc = _get_nc()
    in_maps = _make_in_maps(x, T)
    res = run_bass_kernel_spmd(nc, in_maps, core_ids=list(range(NCORE)))
    return _assemble(x, res.results)


def _ensure_ntff_hook():
    """The agent image's antenv lacks axon_hooks; synthesize it from the
    ctypes NTFF driver in trn_agent_boot so trace=True works."""
    import sys
    import types

    try:
        from antenv.axon_hooks import get_axon_ntff_profile_hook  # noqa: F401

        return
    except ImportError:
        pass
    from trn_agent_boot.trn_boot import _ntff_profile_via_ctypes

    hook = _ntff_profile_via_ctypes("/opt/axon/libaxon_pjrt.so")
    mod = types.ModuleType("antenv.axon_hooks")
    mod.get_axon_ntff_profile_hook = lambda: hook
    mod.set_axon_ntff_profile_hook = lambda h: None
    sys.modules["antenv.axon_hooks"] = mod


def kernel_profiled(x, T, tmpdir=None):
    """Same as kernel() but with NTFF tracing; returns (out, exec_time_ns)."""
    import concourse.bass_utils as bu

    _ensure_ntff_hook()
    bu.upload_artifacts = lambda d: d  # no S3 in this container

    nc = _get_nc()
    in_maps = _make_in_maps(x, T)
    res = bu.run_bass_kernel_spmd(
        nc, in_maps, core_ids=list(range(NCORE)), trace=True, tmpdir=tmpdir
    )
    return _assemble(x, res.results), res.exec_time_ns


# revision 15
# speedup vs baseline: 1.1400x; 1.1123x over previous
"""Trainium2 Bass kernel for nn_MinibatchDiscriminator.

reference:
    M = (x @ T).reshape(B, OUT_F, KD)
    norm[i, j, o] = sum_k |M[i,o,k] - M[j,o,k]|
    oX[j, o] = sum_i exp(-norm[i,j,o])
    out = concat(x, oX, axis=1)

Sharding: batch dim of the j-loop across 8 cores. Each core receives a
batch-rotated copy of x^T (so its own 128 j-rows are always M_T columns
0..127 -- one SPMD program serves all cores), computes the full
M_T = (x_rot @ T)^T in [ok, i] layout on the PE.

Symmetry: exp(-norm) is symmetric in (i, j), so each core only computes
i in [0, 640) local (its own diagonal block, neighbours d=1..3, and the
d=4 block which both endpoint cores compute for their own rows). For
d=1..3 the per-(o, i) column sums over the core's j rows are also
accumulated (tile SACC) and redistributed to the i-owning shards during
host-side assembly; the diagonal block contains both (i,j) orders and
the d=4 block is computed by both endpoints, so neither contributes
column sums.

The L1 abs is computed via the relu identity (the TRN2 tensor_scalar ISA
has no float-abs ALU op, but (add, max) is a legal dual-op pair):

    |d| = 2 relu(d) - d  =>  norm = 2 sum_k relu(d_k) - S_i + S_j,
    S[o, i] = sum_k M[i, o, k]

so generation is ONE dual-op DVE tensor_scalar per chunk
((x + (-M_j)) max 0.0, 4x bf16 mode), the -0.5*S_i correction is its
own T0/T1 matmul pair through an identity selector against a constant
-0.5*S tile, and +S_j enters as the exp bias column with scale=-2.
Both S_i and S_j are read from the same bf16 S values, so they cancel
exactly on the diagonal and exp(0)=1 stays exact.

Chunk 3 has only 16 live ok-rows per jsub, so FOUR j-pairs' worth (8 j
values x 16 rows = 128 partitions) are packed into one gen tile,
regenerated once per 4 pr; per-pr selector variants pick the right
32-row band. The per-group scalar column nmt3big is assembled at setup
with 8 small strided SBUF->SBUF DMAs.

Per j-pair (pr, pr+64), one [128, 640] PSUM tile (rows 0:64 = jsub0's
50 o-rows, 64:128 = jsub1's):
  relu(M_T - M_T[:, j])  one dual-op DVE tensor_scalar per chunk; a
                     rotating ~1.1 tiles/pr go to ACT (Relu activation
                     with per-partition bias) to balance the engines
  k-group reduce     PE matmul with a block-ones selector. jsub0 MMs
                     target col-tile T0 (psum rows 0:64), jsub1 MMs
                     target T1 (rows 64:128); chunk-outer interleaving
                     lets the two 128x64 col-tiles stream concurrently.
                     Both jsubs' 16-row chunk-3 are packed in one gen
                     tile (rows 0:16 / 32:48) via a duplicated column
                     block appended to T, keeping full-128-partition APs
                     so the PE never switches tiling mode mid-loop.
  exp + i-sum        single ACT Exp(scale=-2, bias=S_j col) with accum_out
  sacc               transpose contributions matmul, alternating T0/T1
                     psum halves by pr parity (host adds the halves)

x passthrough is done on the host during assembly (the x-part of the
output is the input x unchanged); the device computes only oX.
"""

import ml_dtypes
import numpy as np

import concourse.bacc as bacc
import concourse.bass as bass
import concourse.mybir as mybir
import concourse.tile as tile

B, IN_F, OUT_F, KD = 1024, 1024, 50, 8
OK = OUT_F * KD  # 400
NCORE = 8
JS = B // NCORE  # 128 rows of the batch per core
P = 128
F32 = mybir.dt.float32
BF16 = mybir.dt.bfloat16

IW = 640  # i-range computed per core (5 of 8 blocks, symmetry)
# matmul free-dim slices of the i-range (<=512 each, psum-bank aligned)
HS = [(0, 512), (512, 640)]
TW = 464  # T input padded: cols 400:464 hold T[:, 384:400] tiled 4x

# (jsub, c) generation tiles routed to ACT per pr (rotating; c=0..2 only,
# the packed chunk-3 tile always stays on DVE). 1 tile/pr balances
# ACT (exp + Relu gens) against DVE (fused relu gens).
def _act_pick(pr):
    return {(pr % 2, 2)}


def _build_nc():
    nc = bacc.Bacc(
        "TRN2",
        target_bir_lowering=False,
        debug=False,
        num_devices=NCORE,
    )
    xT = nc.dram_tensor("xT", [IN_F, IW], BF16, kind="ExternalInput").ap()
    t_in = nc.dram_tensor("T", [IN_F, TW], BF16, kind="ExternalInput").ap()
    sel_in = nc.dram_tensor("sel", [P, 512], BF16, kind="ExternalInput").ap()
    ox_out = nc.dram_tensor("oxpair", [P, 64], F32, kind="ExternalOutput").ap()
    s_out = nc.dram_tensor("sacc", [P, 384], F32, kind="ExternalOutput").ap()

    with tile.TileContext(nc) as tc:
        with (
            tc.tile_pool(name="const", bufs=1) as cpool,
            tc.tile_pool(name="xtp", bufs=1) as xtpool,
            tc.tile_pool(name="agen", bufs=24) as apool,
            tc.tile_pool(name="psn", bufs=3, space=bass.MemorySpace.PSUM) as psn,
            tc.tile_pool(name="esc", bufs=6) as epool,
        ):
            sel_sb = cpool.tile([P, 512], BF16)
            nc.sync.dma_start(out=sel_sb[:], in_=sel_in)

            # spread input loads over several engine DMA queues so the
            # descriptor generation isn't serialized on one sequencer
            dma_engs = [nc.sync, nc.scalar, nc.gpsimd]
            t_sb = []
            xt_sb = []
            for fc in range(8):
                tt = cpool.tile([P, TW], BF16, tag=f"t{fc}")
                dma_engs[fc % 3].dma_start(
                    out=tt[:], in_=t_in[fc * 128 : (fc + 1) * 128, :]
                )
                t_sb.append(tt)
                xtt = xtpool.tile([P, IW], BF16, tag=f"xt{fc}")
                dma_engs[(fc + 1) % 3].dma_start(
                    out=xtt[:, 0:512], in_=xT[fc * 128 : (fc + 1) * 128, 0:512]
                )
                dma_engs[(fc + 2) % 3].dma_start(
                    out=xtt[:, 512:IW], in_=xT[fc * 128 : (fc + 1) * 128, 512:IW]
                )
                xt_sb.append(xtt)

            # M_T chunks [128, 640] in bf16 (+ negated copy for the scalar
            # operands). bf16 is safe: the smallest cross-pair L1 norm is
            # ~50 while exp(-norm) only registers against the exact self
            # term below norm ~16, so +-2 of bf16 noise cannot surface.
            mtb = [cpool.tile([P, IW], BF16, tag=f"mtb{c}", name=f"mtb{c}") for c in range(3)]
            # chunk 3 source tiled 4x down partitions 0:64: row 16t+r of
            # m3big = M3 ok-row r (t = 2q+s indexes the (q, s) j-slot)
            m3big = cpool.tile([64, IW], BF16, tag="m3", name="m3big")
            # negated fp32 copies OF THE BF16 VALUES (exact upcast) for the
            # per-partition scalar/bias operands, which must be fp32; using
            # raw-fp32 M here would break the exact-zero self term.
            nmt32 = [cpool.tile([P, JS], F32, tag=f"nmt32{c}", name=f"nmt32{c}") for c in range(3)]
            # per-group scalar: nmt3big[16t+r, g] = -M3[r, 2g + q + 64 s]
            nmt3big = cpool.tile([64, 32], F32, tag="nmt3b", name="nmt3big")

            for c in range(3):
                lo = c * 128
                for lo2, hi2 in HS:
                    w2 = hi2 - lo2
                    ps = psn.tile([P, 512], F32, tag="psmt", bufs=2)
                    for fc in range(8):
                        for half in range(2):
                            nc.tensor.matmul(
                                ps[64 * half : 64 * half + 64, 0:w2],
                                t_sb[fc][:, lo + 64 * half : lo + 64 * half + 64],
                                xt_sb[fc][:, lo2:hi2],
                                start=(fc == 0),
                                stop=(fc == 7),
                                skip_group_check=True,
                            )
                    if lo2 == 0:
                        nc.scalar.activation(
                            mtb[c][:, lo2:hi2],
                            ps[:, 0:w2],
                            mybir.ActivationFunctionType.Copy,
                            bias=0.0,
                            scale=1.0,
                        )
                    else:
                        nc.vector.tensor_copy(mtb[c][:, lo2:hi2], ps[:, 0:w2])
                nc.vector.tensor_scalar(
                    nmt32[c][:], mtb[c][:, 0:JS], -1.0, None,
                    op0=mybir.AluOpType.mult,
                )
            # chunk 3 build: T cols 400:464 hold T3 tiled 4x
            for lo2, hi2 in HS:
                w2 = hi2 - lo2
                ps = psn.tile([P, 512], F32, tag="psmt", bufs=2)
                for fc in range(8):
                    nc.tensor.matmul(
                        ps[0:64, 0:w2],
                        t_sb[fc][:, 400:464],
                        xt_sb[fc][:, lo2:hi2],
                        start=(fc == 0),
                        stop=(fc == 7),
                    )
                nc.vector.tensor_copy(m3big[:, lo2:hi2], ps[0:64, 0:w2])
            # nmt3big[16t+r, g] = -M3[r, 2g + q + 64 s], t = 2q + s:
            # negate once, then 4 strided partition-shift DMAs
            negm3 = cpool.tile([16, JS], F32, tag="negm3", name="negm3")
            nc.vector.tensor_scalar(
                negm3[:], m3big[0:16, 0:JS], -1.0, None,
                op0=mybir.AluOpType.mult,
            )
            for q in range(2):
                for s in range(2):
                    t = 2 * q + s
                    nc.sync.dma_start(
                        out=nmt3big[16 * t : 16 * t + 16, 0:32],
                        in_=negm3[0:16, q + 64 * s : q + 64 * s + 63 : 2],
                    )

            # S[o, i] = sum_k M[i, o, k] via the selector matmuls (the
            # q=0/T0 chunk-3 selector picks m3big rows 0:16 = M3 once;
            # its identity rows touch rhs partitions >= 64 which m3big
            # does not have, so the lhsT is sliced to 64 rows there)
            psS = psn.tile([P, IW], F32, tag="psn", name="psS")
            for lo2, hi2 in HS:
                for ci, srct in enumerate([mtb[0], mtb[1], mtb[2], m3big]):
                    if ci < 3:
                        wsel = sel_sb[:, 64 * ci : 64 * ci + 64]
                        rhs = srct[:, lo2:hi2]
                    else:
                        wsel = sel_sb[0:64, 192:256]
                        rhs = srct[0:64, lo2:hi2]
                    nc.tensor.matmul(
                        psS[64:128, lo2:hi2],
                        wsel,
                        rhs,
                        start=(ci == 0),
                        stop=(ci == 3),
                    )

            # two alternating 2-pr-group chunk-3 tiles: rows 0:64 hold 4
            # j-slots (regenerated every other pr), rows 64:114 hold the
            # persistent -0.5*S correction (bf16), rest zero
            a3t = []
            for par in range(2):
                a3 = cpool.tile([P, IW], BF16, tag=f"a3_{par}", name=f"a3_{par}")
                nc.vector.memset(a3[:], 0.0)
                a3t.append(a3)
            for lo2, hi2 in HS:
                nc.vector.tensor_scalar(
                    a3t[0][64:114, lo2:hi2], psS[64:114, lo2:hi2], -0.5, None,
                    op0=mybir.AluOpType.mult,
                )
            nc.vector.tensor_copy(a3t[1][64:114, :], a3t[0][64:114, :])
            # exp bias column: sjcol[r, pr] = 2 * sneg_bf16[o(r), j(r, pr)]
            # (= -S_bf[o, j]); rows 64:114 need the j+64 columns, moved
            # across partitions with a small SBUF->SBUF DMA
            sj2 = cpool.tile([P, JS], F32, tag="sj2", name="sj2")
            sjcol = cpool.tile([P, 64], F32, tag="sjcol", name="sjcol")
            nc.vector.memset(sjcol[:], 0.0)
            nc.vector.tensor_scalar(
                sj2[64:114, :], a3t[0][64:114, 0:JS], 2.0, None,
                op0=mybir.AluOpType.mult,
            )
            nc.sync.dma_start(out=sjcol[0:50, :], in_=sj2[64:114, 0:64])
            nc.sync.dma_start(out=sjcol[64:114, :], in_=sj2[64:114, 64:128])

            oxacc = cpool.tile([P, 64], F32)
            psum_s = psn.tile([P, 384], F32, tag="psmt", bufs=2, name="psum_s")

            add = mybir.AluOpType.add
            mx = mybir.AluOpType.max

            # main loop: j-pairs (pr, pr+64) share one [128, IW] psum
            # tile. Software-pipelined 1 deep: generation for pr is
            # emitted BEFORE the matmuls/exp of pr-1 so the ACT-assigned
            # relu gens sit ahead of the stalling exp in ACT's FIFO
            # (breaking the exp -> relu -> matmul -> exp critical cycle).
            def emit_gens(pr):
                act_set = _act_pick(pr)
                av = [[None] * 3 for _ in range(2)]
                for jsub in range(2):
                    j = pr + 64 * jsub
                    for c in range(3):
                        a = apool.tile([P, IW], BF16, tag="A")
                        if (jsub, c) in act_set:
                            nc.scalar.activation(
                                a[:],
                                mtb[c][:],
                                mybir.ActivationFunctionType.Relu,
                                bias=nmt32[c][:, j : j + 1],
                                scale=1.0,
                            )
                        else:
                            nc.vector.tensor_scalar(
                                a[:], mtb[c][:], nmt32[c][:, j : j + 1], 0.0,
                                op0=add, op1=mx,
                            )
                        av[jsub][c] = a
                g = pr // 2
                a3 = a3t[g % 2]
                if pr % 2 == 0:
                    nc.vector.tensor_scalar(
                        a3[0:64, :], m3big[:], nmt3big[:, g : g + 1], 0.0,
                        op0=add, op1=mx,
                    )
                return av, a3

            def emit_reduce(pr, av, a3):
                # k-group reduce: chunk-outer, jsub-inner so consecutive
                # matmuls land on alternating 128x64 col-tiles T0/T1 and
                # stream concurrently. The c=3 matmul also applies the
                # -0.5*S_i correction through the identity selector block.
                ps = psn.tile([P, IW], F32, tag="psn")
                q = pr % 2
                for lo, hi in HS:
                    for ci, c in enumerate([3, 0, 1, 2]):
                        for jsub in range(2):
                            r0 = 64 * jsub
                            if c < 3:
                                w = sel_sb[:, 64 * c : 64 * (c + 1)]
                                rhs = av[jsub][c]
                            else:
                                w = sel_sb[:, 192 + 128 * q + 64 * jsub : 256 + 128 * q + 64 * jsub]
                                rhs = a3
                            nc.tensor.matmul(
                                ps[r0 : r0 + 64, lo:hi],
                                w,
                                rhs[:, lo:hi],
                                start=(ci == 0),
                                stop=(ci == 3),
                                skip_group_check=True,
                            )
                e = epool.tile([P, IW], BF16, tag="E")
                nc.scalar.activation(
                    e[:],
                    ps[:],
                    mybir.ActivationFunctionType.Exp,
                    bias=sjcol[:, pr : pr + 1],
                    scale=-2.0,
                    accum_out=oxacc[:, pr : pr + 1],
                )
                # transpose contributions for the d=1..3 i-blocks: fold the
                # two j-halves and accumulate over all pairs on the PE.
                # Alternate psum halves (col-tiles T0/T1) by pr parity so
                # the sacc matmul overlaps the other tile's chunk chain.
                s0 = 64 * (pr % 2)
                nc.tensor.matmul(
                    psum_s[s0 : s0 + 64, :],
                    sel_sb[:, 448:512],
                    e[:, 128:512],
                    start=(pr < 2),
                    stop=(pr >= 62),
                    skip_group_check=True,
                )

            prev = None
            for pr in range(65):
                cur = emit_gens(pr) if pr < 64 else None
                if prev is not None:
                    emit_reduce(pr - 1, *prev)
                prev = cur

            sacc_sb = cpool.tile([P, 384], F32)
            nc.vector.tensor_copy(sacc_sb[:], psum_s[:])
            nc.sync.dma_start(out=ox_out, in_=oxacc[:])
            nc.sync.dma_start(out=s_out, in_=sacc_sb[:])

    nc.compile()
    return nc


_NC = None


def _get_nc():
    global _NC
    if _NC is None:
        _NC = _build_nc()
    return _NC


def _make_in_maps(x, t):
    x = np.ascontiguousarray(np.asarray(x, dtype=np.float32))
    t16 = np.asarray(t, dtype=np.float32).astype(ml_dtypes.bfloat16)
    tpad = np.zeros((IN_F, TW), dtype=ml_dtypes.bfloat16)
    tpad[:, 0:OK] = t16
    tpad[:, 400:464] = np.tile(t16[:, 384:400], (1, 4))
    tpad = np.ascontiguousarray(tpad)
    xtg = np.ascontiguousarray(x.T.astype(ml_dtypes.bfloat16))
    sel = np.zeros((P, 512), dtype=ml_dtypes.bfloat16)
    # chunks 0..2: chunk c maps partition p (= ok - 128c) to o-row
    # 16c + p // KD of the 64-row psum block
    for c in range(3):
        for g in range(16):
            sel[g * KD : (g + 1) * KD, 64 * c + 16 * c + g] = 1.0
    # chunk-3 variants: for pr%2 == q, the jsub-s matmul picks m3big's
    # 16-row band at 32q + 16s and maps its two o-groups to rows 48:50;
    # every variant also carries the identity block (rows 64:114 -> o)
    # for the -0.5*S_i correction
    for q in range(2):
        for s in range(2):
            base = 32 * q + 16 * s
            region = 192 + 128 * q + 64 * s
            for g in range(2):
                sel[base + g * KD : base + (g + 1) * KD, region + 48 + g] = 1.0
            for o in range(OUT_F):
                sel[64 + o, region + o] = 1.0
    # sacc fold: psum partition p -> column p % 64
    for pp in range(P):
        sel[pp, 448 + (pp % 64)] = 1.0
    in_maps = []
    for c in range(NCORE):
        in_maps.append(
            {
                "xT": np.ascontiguousarray(np.roll(xtg, -c * JS, axis=1)[:, :IW]),
                "T": tpad,
                "sel": sel,
            }
        )
    return in_maps


def _assemble(x, results):
    x = np.asarray(x, dtype=np.float32)
    out = np.empty((B, IN_F + OUT_F), dtype=np.float32)
    out[:, :IN_F] = x
    oX = np.zeros((B, OUT_F), dtype=np.float32)
    for c in range(NCORE):
        r = results[c]
        rows = slice(c * JS, (c + 1) * JS)
        oxp = r["oxpair"]  # [128, 64]: rows 0:50 -> j=pr, rows 64:114 -> j=pr+64
        oX[rows] += np.concatenate(
            [oxp[0:OUT_F, :].T, oxp[64 : 64 + OUT_F, :].T], axis=0
        )
        # transpose contributions: sacc[(parity, o), t] sums exp terms over
        # this core's even/odd j rows for local i = 128 + t (d=1..3 blocks)
        s = r["sacc"]
        s50 = (s[0:OUT_F, :] + s[64 : 64 + OUT_F, :]).T  # [384, 50]
        g0 = (c + 1) * JS
        for blk in range(3):
            gs = (g0 + blk * JS) % B
            oX[gs : gs + JS] += s50[blk * JS : (blk + 1) * JS]
    out[:, IN_F:] = oX
    return out


def kernel(x, T):
    from concourse.bass_utils import run_bass_kernel_spmd

    nc = _get_nc()
    in_maps = _make_in_maps(x, T)
    res = run_bass_kernel_spmd(nc, in_maps, core_ids=list(range(NCORE)))
    return _assemble(x, res.results)


def _ensure_ntff_hook():
    """The agent image's antenv lacks axon_hooks; synthesize it from the
    ctypes NTFF driver in trn_agent_boot so trace=True works."""
    import sys
    import types

    try:
        from antenv.axon_hooks import get_axon_ntff_profile_hook  # noqa: F401

        return
    except ImportError:
        pass
    from trn_agent_boot.trn_boot import _ntff_profile_via_ctypes

    hook = _ntff_profile_via_ctypes("/opt/axon/libaxon_pjrt.so")
    mod = types.ModuleType("antenv.axon_hooks")
    mod.get_axon_ntff_profile_hook = lambda: hook
    mod.set_axon_ntff_profile_hook = lambda h: None
    sys.modules["antenv.axon_hooks"] = mod


def kernel_profiled(x, T, tmpdir=None):
    """Same as kernel() but with NTFF tracing; returns (out, exec_time_ns)."""
    import concourse.bass_utils as bu

    _ensure_ntff_hook()
    bu.upload_artifacts = lambda d: d  # no S3 in this container

    nc = _get_nc()
    in_maps = _make_in_maps(x, T)
    res = bu.run_bass_kernel_spmd(
        nc, in_maps, core_ids=list(range(NCORE)), trace=True, tmpdir=tmpdir
    )
    return _assemble(x, res.results), res.exec_time_ns
